# revision 2
# baseline (speedup 1.0000x reference)
"""MinkowskiEngine deconv+ReLU v2: host-staged X stream + indicator-matmul
scatter, bf16 GEMM, no SWDGE.

Per core (output-partitioned, rows [c*rpc_eff, ...)):
  host sorts the core's K*M/8 pairs by (block, k, out-row); k-pure chunks of
  128 pairs; X = feats rows per chunk, pre-transposed to lhsT layout, streamed
  sequentially (no gather). Device: chunk GEMM (bf16) -> contrib PSUM ->
  ACT copy to SBUF bf16 -> indicator matmuls (lhsT=contrib, rhs=0/1 S built
  on DVE by is_equal(iota, srel)) accumulate out^T tiles in PSUM ->
  ACT bias+ReLU -> sequential DMA out. Host transposes/concats the output.

Structural schedule (chunk/instance counts) shared by all 8 cores; only
tensor contents differ (SPMD single program).
"""
import numpy as np
from contextlib import ExitStack

import concourse.bass as bass
import concourse.bacc as bacc
from concourse import mybir

P = 128
GG = 14          # tiles per block (block = PSUM-resident out^T stripe)
CH = 32          # chunks per X DMA tile (2 MB)
NXB = 3          # x stream buffers
NCSB = 4         # contrib sbuf ring (bank batches)
NSS = 8          # S sbuf slots (batches of SB instances)
SB = 8           # instances per S build
BCH = 4          # chunks per contrib PSUM bank batch
N_CORES = 8

# ---------------------------------------------------------------------------
# host schedule
# ---------------------------------------------------------------------------


def _windows(nb, n_t):
    return [((j * n_t) // nb, min(n_t - 1, ((j + 1) * n_t) // nb))
            for j in range(nb)]


def _greedy(d, ws):
    """d: per-tile pair counts. Returns (ok, takes) with takes[j]
    = [(tile, n), ...]."""
    d = list(d)
    takes = []
    for (lo, hi) in ws:
        room = P
        tk = []
        for t in range(lo, hi + 1):
            n = min(room, d[t])
            if n:
                tk.append((t, n))
                d[t] -= n
                room -= n
        takes.append(tk)
    return sum(d) == 0, takes


def build_schedule(in_map, out_map, n_in, n_out, n_cores=N_CORES):
    K, M = in_map.shape
    rpc_eff = -(-n_out // n_cores)
    n_tiles = -(-rpc_eff // P)
    assert n_tiles % GG == 0, (n_tiles, GG)
    nblk = n_tiles // GG
    rpc = n_tiles * P

    kk = np.repeat(np.arange(K, dtype=np.int64), M)
    irow = in_map.astype(np.int64).ravel()
    orow = out_map.astype(np.int64).ravel()
    core = np.minimum(orow // rpc_eff, n_cores - 1)

    percore = []
    for c in range(n_cores):
        m = core == c
        kc, ic, oc = kk[m], irow[m], orow[m] - c * rpc_eff
        blk = oc // (GG * P)
        order = np.lexsort((oc, kc, blk))
        percore.append((kc[order], ic[order], oc[order], blk[order]))

    # bucket counts and common NB
    cnt = np.zeros((n_cores, nblk, K), np.int64)
    for c in range(n_cores):
        kc, ic, oc, blk = percore[c]
        np.add.at(cnt[c], (blk, kc), 1)
    NB = np.maximum(-(-cnt.max(0) // P), 1)

    # per-core per-bucket tile histograms, bump NB until greedy feasible
    tilehist = np.zeros((n_cores, nblk, K, GG), np.int64)
    for c in range(n_cores):
        kc, ic, oc, blk = percore[c]
        tl = (oc - blk * GG * P) // P
        np.add.at(tilehist[c], (blk, kc, tl), 1)
    for b in range(nblk):
        for k in range(K):
            while True:
                ws = _windows(int(NB[b, k]), GG)
                if all(_greedy(tilehist[c, b, k], ws)[0]
                       for c in range(n_cores)):
                    break
                NB[b, k] += 1

    # shared chunk table
    chunks = []                      # (blk, k, lo, hi)
    for b in range(nblk):
        for k in range(K):
            for (lo, hi) in _windows(int(NB[b, k]), GG):
                chunks.append((b, k, lo, hi))
    nch = len(chunks)

    # shared instance table (for DVE S builds; None = padding) and merged
    # IND groups: one matmul per (chunk, psum-bank run), rhs = s consecutive
    # S tiles. Groups may not straddle an S-batch (SB) boundary.
    insts = []                       # (chunk_id, tile_local) | None
    groups = []                      # (chunk_id, t0, s, i0)
    for ci, (b, k, lo, hi) in enumerate(chunks):
        t = lo
        while t <= hi:
            t1 = min(hi, (t // 4) * 4 + 3)     # end of bank run
            s = t1 - t + 1
            if len(insts) % SB + s > SB:       # pad to S-batch boundary
                insts += [None] * (SB - len(insts) % SB)
            groups.append((ci, t, s, len(insts)))
            for tt in range(t, t1 + 1):
                insts.append((ci, tt))
            t = t1 + 1
    ninst = len(insts)
    # PSUM accumulation groups are per 2KB bank (start zeroes the whole
    # bank): flags keyed by (block, bank), over merged groups
    first = {}
    last = {}
    for g, (ci, t0, s, i0) in enumerate(groups):
        b = chunks[ci][0]
        key = (b, t0 // 4)
        if key not in first:
            first[key] = g
        last[key] = g
    for b in range(nblk):
        for t in range(GG):
            assert (b, t // 4) in first, ("uncovered bank", b, t)

    # per-core slot data: gidx [nch, P] feats row ids, orel_b [nch, P]
    # block-local out row (or -3000 for padding)
    per_core = []
    for c in range(n_cores):
        kc, ic, oc, blk = percore[c]
        gidx = np.zeros((nch, P), np.int64)
        orel = np.full((nch, P), -3000, np.int64)
        # bucket start offsets in the sorted pair list
        bstart = np.zeros((nblk, K), np.int64)
        np.cumsum(cnt[c].ravel()[:-1], out=bstart.ravel()[1:])
        ci = 0
        for b in range(nblk):
            for k in range(K):
                ws = _windows(int(NB[b, k]), GG)
                ok, takes = _greedy(tilehist[c, b, k], ws)
                assert ok
                # per-tile read pointers within this bucket's sorted pairs
                toff = bstart[b, k] + np.concatenate(
                    ([0], np.cumsum(tilehist[c, b, k])[:-1]))
                ptr = toff.copy()
                for j, tk in enumerate(takes):
                    pos = 0
                    for (t, n) in tk:
                        sl = slice(ptr[t], ptr[t] + n)
                        gidx[ci + j, pos:pos + n] = ic[sl]
                        orel[ci + j, pos:pos + n] = oc[sl] - b * GG * P
                        ptr[t] += n
                        pos += n
                ci += len(takes)
        assert ci == nch
        per_core.append((gidx, orel))

    return dict(NB=NB, chunks=chunks, insts=insts, groups=groups,
                first=first, last=last,
                nch=nch, ninst=ninst, nblk=nblk, n_tiles=n_tiles, rpc=rpc,
                rpc_eff=rpc_eff, K=K, per_core=per_core)


# ---------------------------------------------------------------------------
# input packing
# ---------------------------------------------------------------------------


def make_inputs(feats, weight, bias, sched):
    import ml_dtypes
    bf16 = ml_dtypes.bfloat16
    K = sched["K"]
    nch, ninst = sched["nch"], sched["ninst"]
    chunks, insts = sched["chunks"], sched["insts"]
    c_in = feats.shape[1]
    c_out = weight.shape[2]
    assert c_in == 256 and c_out == 128

    f16 = feats.astype(bf16)
    wd = np.ascontiguousarray(
        weight.astype(bf16).reshape(K, 2, P, c_out).transpose(2, 0, 1, 3)
    ).reshape(P, K * 2 * c_out)
    iota = np.tile(np.arange(P, dtype=np.int16), (P, SB))
    biasd = np.tile(bias.astype(np.float32)[:, None], (1, 8))

    nxt = -(-nch // CH)
    nip = -(-ninst // SB) * SB

    in_maps = []
    for (gidx, orel) in sched["per_core"]:
        # X: [nxt*128, CH*256]; [T*128+p, q*256+h*128+j] = f16[gidx[c,j], h*128+p]
        A = f16[gidx]                                  # [nch, j(P), 256]
        A = A.reshape(nch, P, 2, P).transpose(0, 3, 2, 1)   # [c, p, h, j]
        xpad = np.zeros((nxt * CH, P, 2, P), bf16)
        xpad[:nch] = A
        xd = np.ascontiguousarray(
            xpad.reshape(nxt, CH, P, 2, P).transpose(0, 2, 1, 3, 4)
        ).reshape(nxt * P, CH * 2 * P)
        # srel2: [P, nip*2] int16, col 2i,2i+1 = orel - 128*tile for inst i
        srel = np.full((nip, P), -3000, np.int64)
        for i, inst in enumerate(insts):
            if inst is not None:
                ci, t = inst
                srel[i] = orel[ci] - t * P
        srel2 = np.repeat(srel.T.astype(np.int16), 2, axis=1)
        in_maps.append(dict(xd=xd.view(np.uint16), wd=wd.view(np.uint16),
                            srel2=srel2, iota=iota, biasd=biasd))
    return in_maps


# ---------------------------------------------------------------------------
# device program
# ---------------------------------------------------------------------------


def build_program(sched):
    K = sched["K"]
    nch, ninst, nblk = sched["nch"], sched["ninst"], sched["nblk"]
    chunks, insts = sched["chunks"], sched["insts"]
    groups = sched["groups"]
    first, last = sched["first"], sched["last"]
    rpc = sched["rpc"]
    c_out = 128
    nxt = -(-nch // CH)
    nip = -(-ninst // SB) * SB
    nsb = nip // SB
    nbat = -(-nch // BCH)
    BW = GG * P                    # block out^T width (1792)

    grp_of_chunk = [[] for _ in range(nch)]
    for g, (ci, t0, s, i0) in enumerate(groups):
        grp_of_chunk[ci].append(g)

    # ---- symbolic engine sequences for position bookkeeping ----
    # PE: per batch a: MMs (2 per chunk), then merged INDs of batch a-1
    pe_seq = []
    for a in range(nbat + 1):
        if a < nbat:
            for c in range(a * BCH, min((a + 1) * BCH, nch)):
                pe_seq.append(("MM", c))           # one entry = 2 matmuls
        if a >= 1:
            lo, hi = (a - 1) * BCH, min(a * BCH, nch)
            for ci in range(lo, hi):
                for g in grp_of_chunk[ci]:
                    pe_seq.append(("IND", g))
    pe_pos = {}
    pos = 0
    for e in pe_seq:
        pos += 1                     # pe_sem: +1 per MM pair, +1 per IND
        pe_pos[e] = pos              # sem value after entry completes
    pe_total = pos

    pe_pos_mm_hi = {}                # chunk -> pos
    pe_pos_ind = {}                  # group -> pos
    for e in pe_seq:
        if e[0] == "MM":
            pe_pos_mm_hi[e[1]] = pe_pos[e]
        else:
            pe_pos_ind[e[1]] = pe_pos[e]

    # ACT: per batch a: CP(a); after the batch that completes block b's INDs:
    # RELU(b) (4 bank instrs)
    # INDs of chunks in batch a are emitted in PE iteration a+1.
    blk_last_inst = {}               # block -> last group pos
    for b in range(nblk):
        cand = [pe_pos_ind[g] for g, (ci, t0, s, i0) in enumerate(groups)
                if chunks[ci][0] == b]
        blk_last_inst[b] = max(cand)
    # batch index after which block b INDs are done:
    # last chunk of block b:
    blk_last_chunk = {}
    for ci, (b, k, lo, hi) in enumerate(chunks):
        blk_last_chunk[b] = ci
    act_seq = []
    for a in range(nbat + 1):
        if a < nbat:
            act_seq.append(("CP", a))
        for b in range(nblk):
            if blk_last_chunk[b] // BCH == a - 1:
                for bank in range(4):
                    act_seq.append(("RELU", b, bank))
    act_pos = {}
    pos = 0
    for e in act_seq:
        pos += 1
        act_pos[e] = pos
    act_pos_cp = {e[1]: act_pos[e] for e in act_seq if e[0] == "CP"}
    act_pos_relu = {}                # block -> pos of 4th relu
    act_pos_relu_bank = {}           # (block, bank) -> pos
    for e in act_seq:
        if e[0] == "RELU":
            act_pos_relu[e[1]] = act_pos[e]
            act_pos_relu_bank[(e[1], e[2])] = act_pos[e]
    act_total = pos

    # DVE consumer positions: last IND group consuming S batch sb
    sbatch_last_ind = {}
    for g, (ci, t0, s, i0) in enumerate(groups):
        sb = i0 // SB
        sbatch_last_ind[sb] = max(sbatch_last_ind.get(sb, 0), pe_pos_ind[g])
    for sb in range(nsb):            # padding-only batches: no consumer
        if sb not in sbatch_last_ind:
            sbatch_last_ind[sb] = 0

    # S builds all on DVE (walrus rejects tensor_tensor on Pool)
    on_dve = [True for sb in range(nsb)]
    eng_cum = []                     # sb -> (engine_idx, count-within-engine)
    ndve = ngp = 0
    for sb in range(nsb):
        if on_dve[sb]:
            ndve += 1
            eng_cum.append((0, ndve))
        else:
            ngp += 1
            eng_cum.append((1, ngp))
    # highest S batch needed by each block's IND groups
    sb_hi_blk = {}
    for g, (ci, t0, s, i0) in enumerate(groups):
        b = chunks[ci][0]
        sb_hi_blk[b] = max(sb_hi_blk.get(b, 0), (i0 + s - 1) // SB)

    # out-dma count before block b's osb slot (b%2) is free
    nc = bacc.Bacc("TRN2", target_bir_lowering=False, debug=False)
    xd_t = nc.dram_tensor("xd", [nxt * P, CH * 2 * P], mybir.dt.uint16,
                          kind="ExternalInput").ap()
    wd_t = nc.dram_tensor("wd", [P, K * 2 * c_out], mybir.dt.uint16,
                          kind="ExternalInput").ap()
    srel_t = nc.dram_tensor("srel2", [P, nip * 2], mybir.dt.int16,
                            kind="ExternalInput").ap()
    iota_t = nc.dram_tensor("iota", [P, SB * P], mybir.dt.int16,
                            kind="ExternalInput").ap()
    bias_t = nc.dram_tensor("biasd", [P, 8], mybir.dt.float32,
                            kind="ExternalInput").ap()
    outT = nc.dram_tensor("outT", [P, rpc], mybir.dt.bfloat16,
                          kind="ExternalOutput").ap()

    LOAD_TOTAL = 4 * 16

    with ExitStack() as stack:
        block = stack.enter_context(nc.Block())
        load_sem = stack.enter_context(nc.semaphore("load"))
        x_sems = [stack.enter_context(nc.semaphore(f"x{i}"))
                  for i in range(NXB)]
        pe_sem = stack.enter_context(nc.semaphore("pe"))
        act_sem = stack.enter_context(nc.semaphore("act"))
        dve_sem = stack.enter_context(nc.semaphore("dve"))
        gps_sem = stack.enter_context(nc.semaphore("gps"))
        out_sems = [stack.enter_context(nc.semaphore(f"out{i}"))
                    for i in range(2)]

        # PSUM: ops banks 0-3 (out^T stripe), cps banks 4,5 (contrib)
        ops = stack.enter_context(
            nc.psum_tensor("ops", [P, 4, 512], mybir.dt.float32))
        cps = stack.enter_context(
            nc.psum_tensor("cps", [P, 2, BCH, c_out], mybir.dt.float32))

        x_sb = stack.enter_context(
            nc.sbuf_tensor("x_sb", [P, NXB, CH * 2 * P], mybir.dt.bfloat16))
        w_sb = stack.enter_context(
            nc.sbuf_tensor("w_sb", [P, K * 2 * c_out], mybir.dt.bfloat16))
        srel_sb = stack.enter_context(
            nc.sbuf_tensor("srel_sb", [P, nip * 2], mybir.dt.int16))
        iota_sb = stack.enter_context(
            nc.sbuf_tensor("iota_sb", [P, SB * P], mybir.dt.int16))
        bias_sb = stack.enter_context(
            nc.sbuf_tensor("bias_sb", [P, 8], mybir.dt.float32))
        s_sb = stack.enter_context(
            nc.sbuf_tensor("s_sb", [P, NSS, SB * P], mybir.dt.bfloat16))
        c_sb = stack.enter_context(
            nc.sbuf_tensor("c_sb", [P, NCSB, BCH * c_out], mybir.dt.bfloat16))
        o_sb = stack.enter_context(
            nc.sbuf_tensor("o_sb", [P, 2, BW], mybir.dt.bfloat16))

        @block.sync
        def _(sy):
            sy.dma_start(out=w_sb[:].bitcast(mybir.dt.uint16),
                         in_=wd_t[:]).then_inc(load_sem, 16)
            sy.dma_start(out=srel_sb[:], in_=srel_t[:]).then_inc(load_sem, 16)
            sy.dma_start(out=iota_sb[:], in_=iota_t[:]).then_inc(load_sem, 16)
            sy.dma_start(out=bias_sb[:], in_=bias_t[:]).then_inc(load_sem, 16)
            for T in range(nxt):
                if T >= NXB:
                    lc = min((T - NXB + 1) * CH, nch) - 1
                    sy.wait_ge(pe_sem, pe_pos_mm_hi[lc])
                sy.dma_start(out=x_sb[:, T % NXB, :].bitcast(mybir.dt.uint16),
                             in_=xd_t[T * P:(T + 1) * P, :]
                             ).then_inc(x_sems[T % NXB], 16)

        def s_build(eng, sem, sb):
            m = min(SB, max(1, ninst - sb * SB))
            in1 = bass.AP(srel_sb.ap().tensor, sb * SB * 2,
                          [[nip * 2, P], [2, m], [0, P // 2], [1, 2]])
            eng.tensor_tensor(out=s_sb[:, sb % NSS, :m * P],
                              in0=iota_sb[:, :m * P], in1=in1,
                              op=mybir.AluOpType.is_equal).then_inc(sem, 1)

        @block.gpsimd
        def _(gp):
            for b in range(nblk):
                gp.wait_ge(act_sem, act_pos_relu[b])
                gp.dma_start(out=outT[:, b * BW:(b + 1) * BW],
                             in_=o_sb[:, b % 2, :]).then_inc(out_sems[b % 2], 16)

        @block.tensor
        def _(pe):
            pe.wait_ge(load_sem, LOAD_TOTAL)
            for (op, idx) in pe_seq:
                if op == "MM":
                    c = idx
                    b, k, lo, hi = chunks[c]
                    a = c // BCH
                    T = c // CH
                    if c % CH == 0:
                        pe.wait_ge(x_sems[T % NXB], 16 * (T // NXB + 1))
                    if c % BCH == 0 and a >= 2:
                        pe.wait_ge(act_sem, act_pos_cp[a - 2])
                    q = c % BCH
                    qlast = min((a + 1) * BCH, nch) - a * BCH - 1
                    xoff = (c % CH) * 2 * P
                    pe.matmul(out=cps[:, a % 2, q, :],
                              lhsT=x_sb[:, T % NXB, xoff:xoff + P],
                              rhs=w_sb[:, (k * 2) * c_out:(k * 2 + 1) * c_out],
                              start=(q == 0), stop=False)
                    pe.matmul(out=cps[:, a % 2, q, :],
                              lhsT=x_sb[:, T % NXB, xoff + P:xoff + 2 * P],
                              rhs=w_sb[:, (k * 2 + 1) * c_out:(k * 2 + 2) * c_out],
                              start=False, stop=(q == qlast)).then_inc(pe_sem, 1)
                else:
                    g = idx
                    ci, t0, s, i0 = groups[g]
                    b = chunks[ci][0]
                    a = ci // BCH
                    sb = i0 // SB
                    bank = t0 // 4
                    pe.wait_ge(act_sem, act_pos_cp[a])
                    eng, cum = eng_cum[sb]
                    pe.wait_ge(dve_sem if eng == 0 else gps_sem, cum)
                    if first[(b, bank)] == g and b >= 1:
                        pe.wait_ge(act_sem, act_pos_relu_bank[(b - 1, bank)])
                    col = (t0 % 4) * P
                    pe.matmul(out=ops[:, bank, col:col + s * P],
                              lhsT=c_sb[:, a % NCSB,
                                        (ci % BCH) * c_out:(ci % BCH + 1) * c_out],
                              rhs=s_sb[:, sb % NSS,
                                       (i0 % SB) * P:(i0 % SB + s) * P],
                              start=(first[(b, bank)] == g),
                              stop=(last[(b, bank)] == g),
                              ).then_inc(pe_sem, 1)

        @block.scalar
        def _(sc):
            sc.wait_ge(load_sem, LOAD_TOTAL)
            for e in act_seq:
                if e[0] == "CP":
                    a = e[1]
                    n = min((a + 1) * BCH, nch) - a * BCH
                    sc.wait_ge(pe_sem, pe_pos_mm_hi[a * BCH + n - 1])
                    sc.copy(out=c_sb[:, a % NCSB, :n * c_out],
                            in_=cps[:, a % 2, 0:n, :]).then_inc(act_sem, 1)
                else:
                    _, b, bank = e
                    if bank == 0:
                        sc.wait_ge(pe_sem, blk_last_inst[b])
                        if b >= 2:
                            sc.wait_ge(out_sems[b % 2], 16 * (b // 2))
                    w0 = bank * 512
                    w1 = min(BW, w0 + 512)
                    sc.activation(out=o_sb[:, b % 2, w0:w1],
                                  in_=ops[:, bank, 0:w1 - w0],
                                  func=mybir.ActivationFunctionType.Relu,
                                  bias=bias_sb[:, 0:1], scale=1.0
                                  ).then_inc(act_sem, 1)

        @block.vector
        def _(ve):
            ve.wait_ge(load_sem, LOAD_TOTAL)
            for sb in range(nsb):
                if not on_dve[sb]:
                    continue
                if sb >= NSS:
                    ve.wait_ge(pe_sem, sbatch_last_ind[sb - NSS])
                s_build(ve, dve_sem, sb)

    nc.compile()
    return nc


# ---------------------------------------------------------------------------
# entry
# ---------------------------------------------------------------------------

_CACHE = {}


def kernel(feats, weight, bias, in_map, out_map, n_out):
    from concourse.bass_utils import run_bass_kernel_spmd

    feats = np.asarray(feats, dtype=np.float32)
    weight = np.asarray(weight, dtype=np.float32)
    bias = np.asarray(bias, dtype=np.float32)
    in_map = np.asarray(in_map)
    out_map = np.asarray(out_map)
    n_out = int(n_out)
    n_in = feats.shape[0]
    K = weight.shape[0]

    sched = build_schedule(in_map, out_map, n_in, n_out, N_CORES)
    in_maps = make_inputs(feats, weight, bias, sched)

    key = (n_in, n_out, K, sched["nch"], sched["ninst"])
    nc = _CACHE.get(key)
    if nc is None:
        nc = build_program(sched)
        _CACHE[key] = nc

    res = run_bass_kernel_spmd(nc, in_maps, list(range(N_CORES)))
    rpc_eff = sched["rpc_eff"]
    outs = []
    for c in range(N_CORES):
        r = min(rpc_eff, n_out - c * rpc_eff)
        ot = res.results[c]["outT"]              # [128, rpc] bf16
        outs.append(np.asarray(ot[:, :r], dtype=np.float32).T)
    return np.ascontiguousarray(np.concatenate(outs, 0))
